# revision 33
# baseline (speedup 1.0000x reference)
"""MLA (multi-latent attention) Trainium2 kernel, 8-core SPMD, ~399us.

Sharding: tensor-parallel over heads (4 groups of 4 heads) x data-parallel
over batch (2), = 8 cores. Low-rank a-projections are replicated (no real
inter-core links in this environment; collectives are loopback-stubbed);
q_b/kv_b output dims and out_proj input dim are sharded by head. Each core
returns a token-major partial out-projection [n, 2048] in bf16; the host
sums the 4 head-group partials per batch element in f32.

All-bf16 (fp8/DoubleRow measured 1.8x on the PE but fails the 2e-2 max-err
gate: quantization noise anywhere in the q/k/v path produces heavy-tailed
output errors at peaked-attention tokens). On-chip layout is feature-major
so every matmul contracts over the partition dim with natural layouts.

Attention processes HEAD PAIRS per key block: the two heads' scores share a
[128,2,512] psum pair-tile (one Act instruction exponentiates both), and
their 64-row rope matmuls sit adjacent so they occupy disjoint PE quadrants
(rows 0-63 / 64-127) and overlap in the array. The stream is software-
pipelined at three levels: each AV matmul is emitted one iteration late (PE
never waits on exp), each head's denominator/normalize lands in the next
pair's stream, and each qb's out-projection chains interleave into the NEXT
qb's stream. Denominators accumulate on DVE in f32r and reduce via a
1-cyc/col f32r ones-matmul; the diagonal causal-bias matmul covers only its
live columns. Input DMAs are split m-chunk-major in need-order across the
three DGE rings so the first matmul issues ~13us in and phase-1 chains
chase the x stream. No max subtraction (logits are O(6), far from fp32 exp
overflow).
"""

from contextlib import ExitStack

import numpy as np
import ml_dtypes

import concourse.bacc as bacc
import concourse.mybir as mybir
from concourse.tile import TileContext
from concourse import bass_utils

BF16 = mybir.dt.bfloat16
F32 = mybir.dt.float32
F32R = mybir.dt.float32r
NPBF16 = ml_dtypes.bfloat16

EMBED = 2048
HEADS = 16
NOPE = 128
VDIM = 128
ROPE = 64
Q_HEAD = NOPE + ROPE  # 192
KV_RANK = 512
BASE = 10000.0
SCALE = 1.0 / float(np.sqrt(Q_HEAD))
MASK_BIAS = -30000.0

NH = 4          # heads per core
KC = EMBED // 128   # 16 k-chunks of the embedding dim
RC = KV_RANK // 128  # 4 k-chunks of the kv rank

# psum bank budget: pss 2x[128,2,512] (4 banks) + av 2 + pso 2 = 8
_PS_TAGS = {"pss": 2, "av": 2, "pso": 2}
_PS_ROT = ["pss", "pss", "av", "av", "pso", "pso"]


def _emit(nc, n):
    """Trace the per-core kernel (same program on all 8 cores)."""
    TC = n // 512   # token chunks of 512
    NT = n // 128   # token chunks of 128
    AF = mybir.ActivationFunctionType
    OP = mybir.AluOpType

    # ---- DRAM I/O ----
    d_x = nc.dram_tensor("xT", [128, TC, KC, 512], BF16, kind="ExternalInput")
    d_wqa = nc.dram_tensor("wqa", [128, 4, KC, 128], BF16, kind="ExternalInput")
    d_wkva = nc.dram_tensor("wkva", [128, 4, KC, 128], BF16, kind="ExternalInput")
    d_wkpe = nc.dram_tensor("wkpe", [128, KC, 64], BF16, kind="ExternalInput")
    d_wqb = nc.dram_tensor("wqb", [128, RC, NH * Q_HEAD], BF16, kind="ExternalInput")
    d_wk = nc.dram_tensor("wk", [128, RC, NH * NOPE], BF16, kind="ExternalInput")
    d_wv = nc.dram_tensor("wv", [128, RC, NH * VDIM], BF16, kind="ExternalInput")
    d_wout = nc.dram_tensor("wout", [128, NH, EMBED], BF16, kind="ExternalInput")
    d_cos = nc.dram_tensor("cosd", [128, n], BF16, kind="ExternalInput")
    d_sin = nc.dram_tensor("sind", [128, n], BF16, kind="ExternalInput")
    d_mask = nc.dram_tensor("maskd", [128, 4, 512], BF16, kind="ExternalInput")
    d_negeye = nc.dram_tensor("negeye", [128, 128], BF16, kind="ExternalInput")
    d_rotp = nc.dram_tensor("rotp", [128, 128], BF16, kind="ExternalInput")
    d_rotk = nc.dram_tensor("rotk", [64, 128], BF16, kind="ExternalInput")
    d_eyek = nc.dram_tensor("eyek", [64, 128], BF16, kind="ExternalInput")
    d_ones = nc.dram_tensor("onesd", [128, 1], F32R, kind="ExternalInput")
    d_out = nc.dram_tensor("out", [n, EMBED], BF16, kind="ExternalOutput")

    with TileContext(nc) as tc, ExitStack() as st:
        psum = st.enter_context(tc.tile_pool(name="psum", bufs=1, space="PSUM"))
        rot_i = [0]

        def ps_any(name):
            tag = _PS_ROT[rot_i[0] % 6]
            rot_i[0] += 1
            if tag == "pss":
                t = psum.tile([128, 2, 512], F32, tag=tag, bufs=2, name=name)
                return t[:, rot_i[0] % 2, :]
            return psum.tile([128, 512], F32, tag=tag, bufs=_PS_TAGS[tag], name=name)

        def ps_pair(name):
            return psum.tile([128, 2, 512], F32, tag="pss", bufs=2, name=name)

        def ps_tag(tag, name):
            return psum.tile([128, 512], F32, tag=tag, bufs=_PS_TAGS[tag], name=name)

        # ---- mid pool: phase-1 outputs + rope constants ----
        mid = st.enter_context(tc.tile_pool(name="mid", bufs=1))
        t_qa = mid.tile([128, RC, n], BF16)
        t_ckv = mid.tile([128, RC, n], BF16)
        t_kpr = mid.tile([64, n], BF16)  # raw k_pe (pre-rope)
        t_cos = mid.tile([128, n], BF16)
        t_sin = mid.tile([128, n], BF16)
        t_rotp = mid.tile([128, 128], BF16)
        t_rotk = mid.tile([64, 128], BF16)
        t_eyek = mid.tile([64, 128], BF16)

        # ---- phase-2 weights: prefetched during phase 1 ----
        ph2w = st.enter_context(tc.tile_pool(name="ph2w", bufs=1))
        t_wqb = ph2w.tile([128, RC, NH * Q_HEAD], BF16)
        t_wk = ph2w.tile([128, RC, NH * NOPE], BF16)
        t_wv = ph2w.tile([128, RC, NH * VDIM], BF16)

        # ================= phase 1: qa = x@Wqa, ckv = x@Wkva =============
        with tc.tile_pool(name="ph1", bufs=1) as ph1:
            t_x = ph1.tile([128, TC, KC, 512], BF16)
            t_wqa = ph1.tile([128, 4, KC, 128], BF16)
            t_wkva = ph1.tile([128, 4, KC, 128], BF16)
            t_wkpe = ph1.tile([128, KC, 64], BF16)
            # chunk-0 x in 4 sub-DMAs so chain m=0 chases the DMA; weights
            # split likewise; x chunk 1 rides the scalar queue behind them
            nc.sync.dma_start(
                out=t_x[:, 0, 0:4], in_=d_x.ap()[:, 0, 0:4]
            )
            nc.sync.dma_start(
                out=t_x[:, 0, 4:16], in_=d_x.ap()[:, 0, 4:16]
            )
            nc.scalar.dma_start(out=t_wqa[:, 0], in_=d_wqa.ap()[:, 0])
            nc.scalar.dma_start(out=t_wqa[:, 1:4], in_=d_wqa.ap()[:, 1:4])
            nc.sync.dma_start(out=t_wkva[:, 0:2], in_=d_wkva.ap()[:, 0:2])
            nc.sync.dma_start(out=t_wkva[:, 2:4], in_=d_wkva.ap()[:, 2:4])
            nc.sync.dma_start(out=t_wkpe, in_=d_wkpe.ap())
            for s2 in range(2):
                nc.gpsimd.dma_start(
                    out=t_x[:, 1, 8 * s2 : 8 * s2 + 8],
                    in_=d_x.ap()[:, 1, 8 * s2 : 8 * s2 + 8],
                )
            nc.gpsimd.dma_start(out=t_wqb, in_=d_wqb.ap())
            nc.gpsimd.dma_start(out=t_wk, in_=d_wk.ap())
            nc.gpsimd.dma_start(out=t_wv, in_=d_wv.ap())
            nc.gpsimd.dma_start(out=t_cos, in_=d_cos.ap())
            nc.gpsimd.dma_start(out=t_sin, in_=d_sin.ap())
            nc.gpsimd.dma_start(out=t_rotp, in_=d_rotp.ap())
            nc.gpsimd.dma_start(out=t_rotk, in_=d_rotk.ap())
            nc.gpsimd.dma_start(out=t_eyek, in_=d_eyek.ap())

            for t in range(TC):
                ts = slice(t * 512, (t + 1) * 512)
                if t > 1:
                    nc.sync.dma_start(out=t_x[:, t], in_=d_x.ap()[:, t])
                for m in range(4):  # qa chunks
                    ps = ps_any("ps1")
                    for k in range(KC):
                        nc.tensor.matmul(
                            ps,
                            t_wqa[:, m, k, :],
                            t_x[:, t, k, :],
                            start=(k == 0),
                            stop=(k == KC - 1),
                        )
                    nc.scalar.copy(t_qa[:, m, ts], ps)
                for m in range(4):  # compressed kv chunks
                    ps = ps_any("ps2")
                    for k in range(KC):
                        nc.tensor.matmul(
                            ps,
                            t_wkva[:, m, k, :],
                            t_x[:, t, k, :],
                            start=(k == 0),
                            stop=(k == KC - 1),
                        )
                    nc.scalar.copy(t_ckv[:, m, ts], ps)
                # k_pe chunk (64 wide)
                ps = ps_any("ps3")
                for k in range(KC):
                    nc.tensor.matmul(
                        ps[:64],
                        t_wkpe[:, k, :],
                        t_x[:, t, k, :],
                        start=(k == 0),
                        stop=(k == KC - 1),
                    )
                nc.scalar.copy(t_kpr[:, ts], ps[:64])

        # ---- attention-phase persistent tiles (after ph1 frees) ----
        attn_p = st.enter_context(tc.tile_pool(name="attn_p", bufs=1))
        t_qn = attn_p.tile([128, NH, n], BF16)
        t_qpe = attn_p.tile([128, 2, n], BF16)
        t_kn = attn_p.tile([128, NH, n], BF16)
        t_kpe = attn_p.tile([128, n], BF16)
        t_v = attn_p.tile([128, NT, NH * VDIM], BF16)
        t_ao = attn_p.tile([128, NH, n], BF16)
        t_wout = attn_p.tile([128, NH, EMBED], BF16)
        t_mask = attn_p.tile([128, 4, 512], BF16)
        t_negeye = attn_p.tile([128, 128], BF16)
        nc.scalar.dma_start(out=t_wout, in_=d_wout.ap())
        nc.scalar.dma_start(out=t_mask, in_=d_mask.ap())
        nc.scalar.dma_start(out=t_negeye, in_=d_negeye.ap())
        t_ones = attn_p.tile([128, 1], F32R)
        nc.scalar.dma_start(out=t_ones, in_=d_ones.ap())

        # ==== phase 2: q/k/v projections + rope, interleaved per chunk ====
        with tc.tile_pool(name="ropep", bufs=4) as rp:

            def rope_q(g, t):
                ts = slice(t * 512, (t + 1) * 512)
                pr = ps_any("prq")
                nc.tensor.matmul(pr, t_rotp, t_qpe[:, g, ts])
                tt1 = rp.tile([128, 512], F32, tag="tt1", name="tt1")
                tt2 = rp.tile([128, 512], F32, tag="tt2", name="tt2")
                nc.vector.tensor_tensor(tt1, pr, t_sin[:, ts], op=OP.mult)
                nc.vector.tensor_tensor(
                    tt2, t_qpe[:, g, ts], t_cos[:, ts], op=OP.mult
                )
                nc.vector.tensor_tensor(t_qpe[:, g, ts], tt1, tt2, op=OP.add)

            def rope_k(t):
                ts = slice(t * 512, (t + 1) * 512)
                pr = ps_any("prk")
                pd = ps_any("pdk")
                nc.tensor.matmul(pr, t_rotk, t_kpr[:, ts])
                nc.tensor.matmul(pd, t_eyek, t_kpr[:, ts])
                tt1 = rp.tile([128, 512], F32, tag="tt1", name="tt1")
                tt2 = rp.tile([128, 512], F32, tag="tt2", name="tt2")
                nc.vector.tensor_tensor(tt1, pr, t_sin[:, ts], op=OP.mult)
                nc.vector.tensor_tensor(tt2, pd, t_cos[:, ts], op=OP.mult)
                nc.vector.tensor_tensor(t_kpe[:, ts], tt1, tt2, op=OP.add)

            for t in range(TC):
                ts = slice(t * 512, (t + 1) * 512)
                for m in range(6):  # q: 4 nope chunks + 2 pe chunks
                    ps = ps_any("psq")
                    for k in range(RC):
                        nc.tensor.matmul(
                            ps,
                            t_wqb[:, k, m * 128 : (m + 1) * 128],
                            t_qa[:, k, ts],
                            start=(k == 0),
                            stop=(k == RC - 1),
                        )
                    if m < 4:
                        nc.scalar.copy(t_qn[:, m, ts], ps)
                    else:
                        nc.scalar.copy(t_qpe[:, m - 4, ts], ps)
                rope_q(0, t)
                rope_q(1, t)
                for m in range(4):  # k_nope
                    ps = ps_any("psk")
                    for k in range(RC):
                        nc.tensor.matmul(
                            ps,
                            t_wk[:, k, m * 128 : (m + 1) * 128],
                            t_ckv[:, k, ts],
                            start=(k == 0),
                            stop=(k == RC - 1),
                        )
                    nc.scalar.copy(t_kn[:, m, ts], ps)
                rope_k(t)
                for mt in range(4 * t, 4 * t + 4):  # v, token-major
                    ps = ps_any("psv")
                    for k in range(RC):
                        nc.tensor.matmul(
                            ps,
                            t_ckv[:, k, mt * 128 : (mt + 1) * 128],
                            t_wv[:, k, :],
                            start=(k == 0),
                            stop=(k == RC - 1),
                        )
                    nc.vector.tensor_copy(t_v[:, mt, :], ps)

        # =================== attention + out-proj ========================
        # Cross-qb pipeline: each qb's out-projection chains are interleaved
        # into the NEXT qb's score/exp stream, so the PE always has dense
        # independent work and qb-boundary normalize latency is hidden.
        with (
            tc.tile_pool(name="ptp", bufs=6) as ptp,
            tc.tile_pool(name="smallp", bufs=2) as smallp,
            tc.tile_pool(name="otp", bufs=4) as otp,
        ):
            pending_op = []  # out-proj chain closures from the previous qb

            def make_chain(qb, mt, f):
                def go():
                    tok = qb * 512 + mt * 128
                    ps_o = ps_tag("pso", "pso")
                    for h4 in range(NH):
                        nc.tensor.matmul(
                            ps_o,
                            t_ao[:, h4, tok : tok + 128],
                            t_wout[:, h4, f * 512 : (f + 1) * 512],
                            start=(h4 == 0),
                            stop=(h4 == NH - 1),
                        )
                    ot = otp.tile([128, 512], BF16, tag="ot", name="ot")
                    nc.scalar.copy(ot, ps_o)
                    nc.sync.dma_start(
                        out=d_out.ap()[tok : tok + 128, f * 512 : (f + 1) * 512],
                        in_=ot,
                    )
                return go

            for qb in range(TC):
                qs = slice(qb * 512, (qb + 1) * 512)
                nkb = 4 * qb + 4
                stream = [(hp, kb) for hp in range(2) for kb in range(nkb)]
                ps_avs = {}
                accs = {}
                pend_av = []     # [(h, kb, pt)] awaiting av matmuls
                pend_post = []   # [(h, flush_idx)] delayed den/normalize

                def emit_post(h):
                    ps_den = ps_tag("pso", "psden")
                    nc.tensor.matmul(ps_den[:1], t_ones, accs[h])
                    rec = smallp.tile([1, 512], F32, tag="rec", name="rec")
                    nc.vector.reciprocal_approx_fast(rec, ps_den[:1])
                    bc = smallp.tile([128, 512], F32, tag="bc", name="bc")
                    nc.gpsimd.partition_broadcast(bc, rec)
                    nc.vector.tensor_tensor(
                        t_ao[:, h, qs], ps_avs[h], bc, op=OP.mult
                    )

                def emit_av(h, kb, pt):
                    nc.tensor.matmul(
                        ps_avs[h],
                        t_v[:, kb, h * VDIM : (h + 1) * VDIM],
                        pt,
                        start=(kb == 0),
                        stop=(kb == nkb - 1),
                    )

                for idx, (hp, kb) in enumerate(stream):
                    h0, h1 = 2 * hp, 2 * hp + 1
                    g = hp
                    if kb == 0:
                        ps_avs[h0] = ps_tag("av", "psav")
                        ps_avs[h1] = ps_tag("av", "psav")
                    ks = slice(kb * 128, (kb + 1) * 128)
                    diag = kb >= 4 * qb
                    ps_p = ps_pair("pssp")
                    ps_a = ps_p[:, 0, :]
                    ps_b = ps_p[:, 1, :]
                    nc.tensor.matmul(
                        ps_a, t_kn[:, h0, ks], t_qn[:, h0, qs],
                        start=True, stop=False,
                    )
                    nc.tensor.matmul(
                        ps_b, t_kn[:, h1, ks], t_qn[:, h1, qs],
                        start=True, stop=False,
                    )
                    # adjacent 64-row rope matmuls occupy disjoint PE
                    # quadrants (rows 0-63 / 64-127) and overlap in the array
                    nc.tensor.matmul(
                        ps_a, t_kpe[0:64, ks], t_qpe[0:64, g, qs],
                        start=False, stop=True,
                    )
                    nc.tensor.matmul(
                        ps_b, t_kpe[64:128, ks], t_qpe[64:128, g, qs],
                        start=False, stop=True,
                    )
                    if diag:  # causal bias: -30000 * U_r over live columns
                        r = kb - 4 * qb
                        nc.tensor.matmul(
                            ps_a[:, : (r + 1) * 128],
                            t_negeye,
                            t_mask[:, r, : (r + 1) * 128],
                            start=False, stop=True,
                            skip_group_check=True,
                        )
                        nc.tensor.matmul(
                            ps_b[:, : (r + 1) * 128],
                            t_negeye,
                            t_mask[:, r, : (r + 1) * 128],
                            start=False, stop=True,
                            skip_group_check=True,
                        )
                    while pend_av:
                        emit_av(*pend_av.pop(0))
                    while pend_post and pend_post[0][1] <= idx:
                        emit_post(pend_post.pop(0)[0])
                    if pending_op:
                        pending_op.pop(0)()
                    ptp2 = ptp.tile([128, 2, 512], BF16, tag="pt", name="ptp2")
                    pt0 = ptp2[:, 0, :]
                    pt1 = ptp2[:, 1, :]
                    nc.scalar.activation(
                        ptp2[:, 0:2, :], ps_p[:, 0:2, :], AF.Exp, scale=SCALE
                    )
                    if kb == 0:
                        accs[h0] = smallp.tile(
                            [128, 512], F32R, tag="acc", name="acc"
                        )
                        accs[h1] = smallp.tile(
                            [128, 512], F32R, tag="acc", name="acc"
                        )
                        nc.vector.tensor_copy(accs[h0], pt0)
                        nc.vector.tensor_copy(accs[h1], pt1)
                    else:
                        nc.vector.tensor_tensor(accs[h0], accs[h0], pt0, op=OP.add)
                        nc.vector.tensor_tensor(accs[h1], accs[h1], pt1, op=OP.add)
                    pend_av.append((h0, kb, pt0))
                    pend_av.append((h1, kb, pt1))
                    if kb == nkb - 1:
                        pend_post.append((h0, idx + 1))
                        pend_post.append((h1, idx + 1))
                while pend_av:
                    emit_av(*pend_av.pop(0))
                while pend_post:
                    emit_post(pend_post.pop(0)[0])
                while pending_op:
                    pending_op.pop(0)()
                pending_op = [
                    make_chain(qb, mt, f) for mt in range(4) for f in range(4)
                ]
            while pending_op:
                pending_op.pop(0)()
    return nc


_NC_CACHE = {}


def build_mla(n=2048):
    if n not in _NC_CACHE:
        nc = bacc.Bacc(
            "TRN2",
            target_bir_lowering=False,
            debug=False,
            enable_asserts=False,
        )
        _emit(nc, n)
        nc.compile()
        _NC_CACHE[n] = nc
    return _NC_CACHE[n]


def make_host_inputs(x, Wqa, Wqb, Wkva, Wkvb, Wout, n):
    """Build the 8 per-core input maps (host-side sharding)."""
    # rope tables
    theta = BASE ** (-2.0 * np.arange(ROPE // 2, dtype=np.float32) / ROPE)
    pos = np.arange(n, dtype=np.float32)
    ang = pos[:, None] * theta[None, :]  # [n, 32]
    cos64 = np.repeat(np.cos(ang).T, 2, axis=0).astype(np.float32)  # [64, n]
    sin64 = np.repeat(np.sin(ang).T, 2, axis=0).astype(np.float32)
    cosd = np.tile(cos64, (2, 1))  # [128, n]
    sind = np.tile(sin64, (2, 1))

    kp = np.arange(128)[:, None, None]
    r = np.arange(4)[None, :, None]
    qf = np.arange(512)[None, None, :]
    # U_r: 1.0 where EXCLUDED (future) -> biased by -30000 before exp
    maskd = (qf < r * 128 + kp).astype(NPBF16)
    negeye = (MASK_BIAS * np.eye(128, dtype=np.float32)).astype(NPBF16)

    rot64 = np.zeros((64, 64), np.float32)
    for i in range(32):
        rot64[2 * i + 1, 2 * i] = -1.0
        rot64[2 * i, 2 * i + 1] = 1.0
    rotp = np.zeros((128, 128), np.float32)
    rotp[:64, :64] = rot64
    rotp[64:, 64:] = rot64
    rotk = np.hstack([rot64, rot64])
    eyek = np.hstack([np.eye(64, dtype=np.float32), np.eye(64, dtype=np.float32)])

    def prelay(w, kc):
        # [kc*128, m] -> [128, kc, m] partition-major, contiguous
        return np.ascontiguousarray(
            w.reshape(kc, 128, w.shape[1]).transpose(1, 0, 2)
        ).astype(NPBF16)

    def mmajor(w):
        # [128, KC, 4*128] -> [128, 4, KC, 128]
        return np.ascontiguousarray(
            w.reshape(128, KC, 4, 128).transpose(0, 2, 1, 3)
        )

    wkva_l = prelay(Wkva, KC)
    shared = {
        "wqa": mmajor(prelay(Wqa, KC).astype(np.float32)).astype(NPBF16),
        "wkva": mmajor(wkva_l[:, :, :512].astype(np.float32)).astype(NPBF16),
        "wkpe": np.ascontiguousarray(wkva_l[:, :, 512:576]),
        "cosd": cosd.astype(NPBF16),
        "sind": sind.astype(NPBF16),
        "maskd": maskd,
        "negeye": negeye,
        "onesd": np.ones((128, 1), np.float32),
        "rotp": rotp.astype(NPBF16),
        "rotk": rotk.astype(NPBF16),
        "eyek": eyek.astype(NPBF16),
    }
    Wqb_r = Wqb.reshape(512, HEADS, Q_HEAD)
    Wkvb_r = Wkvb.reshape(KV_RANK, HEADS, NOPE + VDIM)
    Wout_r = Wout.reshape(HEADS, VDIM, EMBED)

    in_maps = []
    TC = n // 512
    # x[be].T -> [128, TC, KC, 512]: f=(c,p), t=(tb,tt)
    xT = [
        np.ascontiguousarray(
            x[be].T.reshape(KC, 128, TC, 512).transpose(1, 2, 0, 3)
        ).astype(NPBF16)
        for be in range(x.shape[0])
    ]
    for c in range(8):
        be, hg = c // 4, c % 4
        hsel = slice(4 * hg, 4 * hg + NH)
        wqb = prelay(
            np.concatenate(
                [
                    Wqb_r[:, hsel, :NOPE].reshape(512, NH * NOPE),
                    Wqb_r[:, hsel, NOPE:].reshape(512, NH * ROPE),
                ],
                axis=1,
            ),
            RC,
        )
        in_maps.append(
            {
                **shared,
                "xT": xT[be],
                "wqb": wqb,
                "wk": prelay(Wkvb_r[:, hsel, :NOPE].reshape(512, NH * NOPE), RC),
                "wv": prelay(Wkvb_r[:, hsel, NOPE:].reshape(512, NH * VDIM), RC),
                "wout": prelay(Wout_r[hsel].reshape(NH * VDIM, EMBED), NH),
            }
        )
    return in_maps


def kernel(x, Wqa, Wqb, Wkva, Wkvb, Wout, _trace=False):
    x = np.asarray(x)
    b, n, _ = x.shape
    nc = build_mla(n)
    in_maps = make_host_inputs(
        np.asarray(x),
        np.asarray(Wqa),
        np.asarray(Wqb),
        np.asarray(Wkva),
        np.asarray(Wkvb),
        np.asarray(Wout),
        n,
    )
    res = bass_utils.run_bass_kernel_spmd(
        nc, in_maps, core_ids=list(range(8)), trace=_trace
    )
    out = np.zeros((b, n, EMBED), np.float32)
    for c in range(8):
        out[c // 4] += res.results[c]["out"].astype(np.float32)
    if _trace:
        kernel.last_results = res
    return out


# revision 34
# speedup vs baseline: 1.0062x; 1.0062x over previous
"""MLA (multi-latent attention) Trainium2 kernel, 8-core SPMD, ~399us.

Sharding: tensor-parallel over heads (4 groups of 4 heads) x data-parallel
over batch (2), = 8 cores. Low-rank a-projections are replicated (no real
inter-core links in this environment; collectives are loopback-stubbed);
q_b/kv_b output dims and out_proj input dim are sharded by head. Each core
returns a token-major partial out-projection [n, 2048] in bf16; the host
sums the 4 head-group partials per batch element in f32.

All-bf16 (fp8/DoubleRow measured 1.8x on the PE but fails the 2e-2 max-err
gate: quantization noise anywhere in the q/k/v path produces heavy-tailed
output errors at peaked-attention tokens). On-chip layout is feature-major
so every matmul contracts over the partition dim with natural layouts.

Attention processes HEAD PAIRS per key block: the two heads' scores share a
[128,2,512] psum pair-tile (one Act instruction exponentiates both), and
their 64-row rope matmuls sit adjacent so they occupy disjoint PE quadrants
(rows 0-63 / 64-127) and overlap in the array. The stream is software-
pipelined at three levels: each AV matmul is emitted one iteration late (PE
never waits on exp), each head's denominator/normalize lands in the next
pair's stream, and each qb's out-projection chains interleave into the NEXT
qb's stream. Denominators accumulate on DVE in f32r and reduce via a
1-cyc/col f32r ones-matmul; the diagonal causal-bias matmul covers only its
live columns. Input DMAs are split m-chunk-major in need-order across the
three DGE rings so the first matmul issues ~13us in and phase-1 chains
chase the x stream. No max subtraction (logits are O(6), far from fp32 exp
overflow).
"""

from contextlib import ExitStack

import numpy as np
import ml_dtypes

import concourse.bacc as bacc
import concourse.mybir as mybir
from concourse.tile import TileContext
from concourse import bass_utils

BF16 = mybir.dt.bfloat16
F32 = mybir.dt.float32
F32R = mybir.dt.float32r
NPBF16 = ml_dtypes.bfloat16

EMBED = 2048
HEADS = 16
NOPE = 128
VDIM = 128
ROPE = 64
Q_HEAD = NOPE + ROPE  # 192
KV_RANK = 512
BASE = 10000.0
SCALE = 1.0 / float(np.sqrt(Q_HEAD))
MASK_BIAS = -30000.0

NH = 4          # heads per core
KC = EMBED // 128   # 16 k-chunks of the embedding dim
RC = KV_RANK // 128  # 4 k-chunks of the kv rank

# psum bank budget: pss 2x[128,2,512] (4 banks) + av 2 + pso 2 = 8
_PS_TAGS = {"pss": 2, "av": 2, "pso": 2}
_PS_ROT = ["pss", "pss", "av", "av", "pso", "pso"]


def _emit(nc, n):
    """Trace the per-core kernel (same program on all 8 cores)."""
    TC = n // 512   # token chunks of 512
    NT = n // 128   # token chunks of 128
    AF = mybir.ActivationFunctionType
    OP = mybir.AluOpType

    # ---- DRAM I/O ----
    d_x = nc.dram_tensor("xT", [128, TC, KC, 512], BF16, kind="ExternalInput")
    d_wqa = nc.dram_tensor("wqa", [128, 4, KC, 128], BF16, kind="ExternalInput")
    d_wkva = nc.dram_tensor("wkva", [128, 4, KC, 128], BF16, kind="ExternalInput")
    d_wkpe = nc.dram_tensor("wkpe", [128, KC, 64], BF16, kind="ExternalInput")
    d_wqb = nc.dram_tensor("wqb", [128, RC, NH * Q_HEAD], BF16, kind="ExternalInput")
    d_wk = nc.dram_tensor("wk", [128, RC, NH * NOPE], BF16, kind="ExternalInput")
    d_wv = nc.dram_tensor("wv", [128, RC, NH * VDIM], BF16, kind="ExternalInput")
    d_wout = nc.dram_tensor("wout", [128, NH, EMBED], BF16, kind="ExternalInput")
    d_cos = nc.dram_tensor("cosd", [128, n], BF16, kind="ExternalInput")
    d_sin = nc.dram_tensor("sind", [128, n], BF16, kind="ExternalInput")
    d_mask = nc.dram_tensor("maskd", [128, 4, 512], BF16, kind="ExternalInput")
    d_negeye = nc.dram_tensor("negeye", [128, 128], BF16, kind="ExternalInput")
    d_rotp = nc.dram_tensor("rotp", [128, 128], BF16, kind="ExternalInput")
    d_rotk = nc.dram_tensor("rotk", [64, 128], BF16, kind="ExternalInput")
    d_eyek = nc.dram_tensor("eyek", [64, 128], BF16, kind="ExternalInput")
    d_ones = nc.dram_tensor("onesd", [128, 1], F32R, kind="ExternalInput")
    d_out = nc.dram_tensor("out", [n, EMBED], BF16, kind="ExternalOutput")

    with TileContext(nc) as tc, ExitStack() as st:
        psum = st.enter_context(tc.tile_pool(name="psum", bufs=1, space="PSUM"))
        rot_i = [0]

        def ps_any(name):
            tag = _PS_ROT[rot_i[0] % 6]
            rot_i[0] += 1
            if tag == "pss":
                t = psum.tile([128, 2, 512], F32, tag=tag, bufs=2, name=name)
                return t[:, rot_i[0] % 2, :]
            return psum.tile([128, 512], F32, tag=tag, bufs=_PS_TAGS[tag], name=name)

        def ps_pair(name):
            return psum.tile([128, 2, 512], F32, tag="pss", bufs=2, name=name)

        def ps_tag(tag, name):
            return psum.tile([128, 512], F32, tag=tag, bufs=_PS_TAGS[tag], name=name)

        # ---- mid pool: phase-1 outputs + rope constants ----
        mid = st.enter_context(tc.tile_pool(name="mid", bufs=1))
        t_qa = mid.tile([128, RC, n], BF16)
        t_ckv = mid.tile([128, RC, n], BF16)
        t_kpr = mid.tile([64, n], BF16)  # raw k_pe (pre-rope)
        t_cos = mid.tile([128, n], BF16)
        t_sin = mid.tile([128, n], BF16)
        t_rotp = mid.tile([128, 128], BF16)
        t_rotk = mid.tile([64, 128], BF16)
        t_eyek = mid.tile([64, 128], BF16)

        # ---- phase-2 weights: prefetched during phase 1 ----
        ph2w = st.enter_context(tc.tile_pool(name="ph2w", bufs=1))
        t_wqb = ph2w.tile([128, RC, NH * Q_HEAD], BF16)
        t_wk = ph2w.tile([128, RC, NH * NOPE], BF16)
        t_wv = ph2w.tile([128, RC, NH * VDIM], BF16)

        # ================= phase 1: qa = x@Wqa, ckv = x@Wkva =============
        with tc.tile_pool(name="ph1", bufs=1) as ph1:
            t_x = ph1.tile([128, TC, KC, 512], BF16)
            t_wqa = ph1.tile([128, 4, KC, 128], BF16)
            t_wkva = ph1.tile([128, 4, KC, 128], BF16)
            t_wkpe = ph1.tile([128, KC, 64], BF16)
            # chunk-0 x in 4 sub-DMAs so chain m=0 chases the DMA; weights
            # split likewise; x chunk 1 rides the scalar queue behind them
            nc.sync.dma_start(
                out=t_x[:, 0, 0:4], in_=d_x.ap()[:, 0, 0:4]
            )
            nc.sync.dma_start(
                out=t_x[:, 0, 4:16], in_=d_x.ap()[:, 0, 4:16]
            )
            nc.scalar.dma_start(out=t_wqa[:, 0], in_=d_wqa.ap()[:, 0])
            nc.scalar.dma_start(out=t_wqa[:, 1:4], in_=d_wqa.ap()[:, 1:4])
            nc.sync.dma_start(out=t_wkva[:, 0:2], in_=d_wkva.ap()[:, 0:2])
            nc.sync.dma_start(out=t_wkva[:, 2:4], in_=d_wkva.ap()[:, 2:4])
            nc.sync.dma_start(out=t_wkpe, in_=d_wkpe.ap())
            for s2 in range(2):
                nc.gpsimd.dma_start(
                    out=t_x[:, 1, 8 * s2 : 8 * s2 + 8],
                    in_=d_x.ap()[:, 1, 8 * s2 : 8 * s2 + 8],
                )
            nc.gpsimd.dma_start(out=t_wqb, in_=d_wqb.ap())
            nc.gpsimd.dma_start(out=t_wk, in_=d_wk.ap())
            nc.gpsimd.dma_start(out=t_wv, in_=d_wv.ap())
            nc.gpsimd.dma_start(out=t_cos, in_=d_cos.ap())
            nc.gpsimd.dma_start(out=t_sin, in_=d_sin.ap())
            nc.gpsimd.dma_start(out=t_rotp, in_=d_rotp.ap())
            nc.gpsimd.dma_start(out=t_rotk, in_=d_rotk.ap())
            nc.gpsimd.dma_start(out=t_eyek, in_=d_eyek.ap())

            for t in range(TC):
                ts = slice(t * 512, (t + 1) * 512)
                if t > 1:
                    nc.sync.dma_start(out=t_x[:, t], in_=d_x.ap()[:, t])
                if t == 0:
                    # block-k-outer: all 4 qa chains chase the x/wqa stream
                    pa = ps_pair("ka")
                    pb = ps_pair("kb")
                    qslots = [pa[:, 0, :], pa[:, 1, :], pb[:, 0, :], pb[:, 1, :]]
                    for kb4 in range(0, KC, 4):
                        for m in range(4):
                            for k in range(kb4, kb4 + 4):
                                nc.tensor.matmul(
                                    qslots[m],
                                    t_wqa[:, m, k, :],
                                    t_x[:, t, k, :],
                                    start=(k == 0),
                                    stop=(k == KC - 1),
                                )
                    for m in range(4):
                        nc.scalar.copy(t_qa[:, m, ts], qslots[m])
                else:
                    for m in range(4):  # qa chunks
                        ps = ps_any("ps1")
                        for k in range(KC):
                            nc.tensor.matmul(
                                ps,
                                t_wqa[:, m, k, :],
                                t_x[:, t, k, :],
                                start=(k == 0),
                                stop=(k == KC - 1),
                            )
                        nc.scalar.copy(t_qa[:, m, ts], ps)
                for m in range(4):  # compressed kv chunks
                    ps = ps_any("ps2")
                    for k in range(KC):
                        nc.tensor.matmul(
                            ps,
                            t_wkva[:, m, k, :],
                            t_x[:, t, k, :],
                            start=(k == 0),
                            stop=(k == KC - 1),
                        )
                    nc.scalar.copy(t_ckv[:, m, ts], ps)
                # k_pe chunk (64 wide)
                ps = ps_any("ps3")
                for k in range(KC):
                    nc.tensor.matmul(
                        ps[:64],
                        t_wkpe[:, k, :],
                        t_x[:, t, k, :],
                        start=(k == 0),
                        stop=(k == KC - 1),
                    )
                nc.scalar.copy(t_kpr[:, ts], ps[:64])

        # ---- attention-phase persistent tiles (after ph1 frees) ----
        attn_p = st.enter_context(tc.tile_pool(name="attn_p", bufs=1))
        t_qn = attn_p.tile([128, NH, n], BF16)
        t_qpe = attn_p.tile([128, 2, n], BF16)
        t_kn = attn_p.tile([128, NH, n], BF16)
        t_kpe = attn_p.tile([128, n], BF16)
        t_v = attn_p.tile([128, NT, NH * VDIM], BF16)
        t_ao = attn_p.tile([128, NH, n], BF16)
        t_wout = attn_p.tile([128, NH, EMBED], BF16)
        t_mask = attn_p.tile([128, 4, 512], BF16)
        t_negeye = attn_p.tile([128, 128], BF16)
        nc.scalar.dma_start(out=t_wout, in_=d_wout.ap())
        nc.scalar.dma_start(out=t_mask, in_=d_mask.ap())
        nc.scalar.dma_start(out=t_negeye, in_=d_negeye.ap())
        t_ones = attn_p.tile([128, 1], F32R)
        nc.scalar.dma_start(out=t_ones, in_=d_ones.ap())

        # ==== phase 2: q/k/v projections + rope, interleaved per chunk ====
        with tc.tile_pool(name="ropep", bufs=4) as rp:

            def rope_q(g, t):
                ts = slice(t * 512, (t + 1) * 512)
                pr = ps_any("prq")
                nc.tensor.matmul(pr, t_rotp, t_qpe[:, g, ts])
                tt1 = rp.tile([128, 512], F32, tag="tt1", name="tt1")
                tt2 = rp.tile([128, 512], F32, tag="tt2", name="tt2")
                nc.vector.tensor_tensor(tt1, pr, t_sin[:, ts], op=OP.mult)
                nc.vector.tensor_tensor(
                    tt2, t_qpe[:, g, ts], t_cos[:, ts], op=OP.mult
                )
                nc.vector.tensor_tensor(t_qpe[:, g, ts], tt1, tt2, op=OP.add)

            def rope_k(t):
                ts = slice(t * 512, (t + 1) * 512)
                pr = ps_any("prk")
                pd = ps_any("pdk")
                nc.tensor.matmul(pr, t_rotk, t_kpr[:, ts])
                nc.tensor.matmul(pd, t_eyek, t_kpr[:, ts])
                tt1 = rp.tile([128, 512], F32, tag="tt1", name="tt1")
                tt2 = rp.tile([128, 512], F32, tag="tt2", name="tt2")
                nc.vector.tensor_tensor(tt1, pr, t_sin[:, ts], op=OP.mult)
                nc.vector.tensor_tensor(tt2, pd, t_cos[:, ts], op=OP.mult)
                nc.vector.tensor_tensor(t_kpe[:, ts], tt1, tt2, op=OP.add)

            for t in range(TC):
                ts = slice(t * 512, (t + 1) * 512)
                for m in range(6):  # q: 4 nope chunks + 2 pe chunks
                    ps = ps_any("psq")
                    for k in range(RC):
                        nc.tensor.matmul(
                            ps,
                            t_wqb[:, k, m * 128 : (m + 1) * 128],
                            t_qa[:, k, ts],
                            start=(k == 0),
                            stop=(k == RC - 1),
                        )
                    if m < 4:
                        nc.scalar.copy(t_qn[:, m, ts], ps)
                    else:
                        nc.scalar.copy(t_qpe[:, m - 4, ts], ps)
                rope_q(0, t)
                rope_q(1, t)
                for m in range(4):  # k_nope
                    ps = ps_any("psk")
                    for k in range(RC):
                        nc.tensor.matmul(
                            ps,
                            t_wk[:, k, m * 128 : (m + 1) * 128],
                            t_ckv[:, k, ts],
                            start=(k == 0),
                            stop=(k == RC - 1),
                        )
                    nc.scalar.copy(t_kn[:, m, ts], ps)
                rope_k(t)
                for mt in range(4 * t, 4 * t + 4):  # v, token-major
                    ps = ps_any("psv")
                    for k in range(RC):
                        nc.tensor.matmul(
                            ps,
                            t_ckv[:, k, mt * 128 : (mt + 1) * 128],
                            t_wv[:, k, :],
                            start=(k == 0),
                            stop=(k == RC - 1),
                        )
                    nc.vector.tensor_copy(t_v[:, mt, :], ps)

        # =================== attention + out-proj ========================
        # Cross-qb pipeline: each qb's out-projection chains are interleaved
        # into the NEXT qb's score/exp stream, so the PE always has dense
        # independent work and qb-boundary normalize latency is hidden.
        with (
            tc.tile_pool(name="ptp", bufs=6) as ptp,
            tc.tile_pool(name="smallp", bufs=2) as smallp,
            tc.tile_pool(name="otp", bufs=4) as otp,
        ):
            pending_op = []  # out-proj chain closures from the previous qb

            def make_chain(qb, mt, f):
                def go():
                    tok = qb * 512 + mt * 128
                    ps_o = ps_tag("pso", "pso")
                    for h4 in range(NH):
                        nc.tensor.matmul(
                            ps_o,
                            t_ao[:, h4, tok : tok + 128],
                            t_wout[:, h4, f * 512 : (f + 1) * 512],
                            start=(h4 == 0),
                            stop=(h4 == NH - 1),
                        )
                    ot = otp.tile([128, 512], BF16, tag="ot", name="ot")
                    nc.scalar.copy(ot, ps_o)
                    nc.sync.dma_start(
                        out=d_out.ap()[tok : tok + 128, f * 512 : (f + 1) * 512],
                        in_=ot,
                    )
                return go

            for qb in range(TC):
                qs = slice(qb * 512, (qb + 1) * 512)
                nkb = 4 * qb + 4
                stream = [(hp, kb) for hp in range(2) for kb in range(nkb)]
                ps_avs = {}
                accs = {}
                pend_av = []     # [(h, kb, pt)] awaiting av matmuls
                pend_post = []   # [(h, flush_idx)] delayed den/normalize

                def emit_post(h):
                    ps_den = ps_tag("pso", "psden")
                    nc.tensor.matmul(ps_den[:1], t_ones, accs[h])
                    rec = smallp.tile([1, 512], F32, tag="rec", name="rec")
                    nc.vector.reciprocal_approx_fast(rec, ps_den[:1])
                    bc = smallp.tile([128, 512], F32, tag="bc", name="bc")
                    nc.gpsimd.partition_broadcast(bc, rec)
                    nc.vector.tensor_tensor(
                        t_ao[:, h, qs], ps_avs[h], bc, op=OP.mult
                    )

                def emit_av(h, kb, pt):
                    nc.tensor.matmul(
                        ps_avs[h],
                        t_v[:, kb, h * VDIM : (h + 1) * VDIM],
                        pt,
                        start=(kb == 0),
                        stop=(kb == nkb - 1),
                    )

                for idx, (hp, kb) in enumerate(stream):
                    h0, h1 = 2 * hp, 2 * hp + 1
                    g = hp
                    if kb == 0:
                        ps_avs[h0] = ps_tag("av", "psav")
                        ps_avs[h1] = ps_tag("av", "psav")
                    ks = slice(kb * 128, (kb + 1) * 128)
                    diag = kb >= 4 * qb
                    ps_p = ps_pair("pssp")
                    ps_a = ps_p[:, 0, :]
                    ps_b = ps_p[:, 1, :]
                    nc.tensor.matmul(
                        ps_a, t_kn[:, h0, ks], t_qn[:, h0, qs],
                        start=True, stop=False,
                    )
                    nc.tensor.matmul(
                        ps_b, t_kn[:, h1, ks], t_qn[:, h1, qs],
                        start=True, stop=False,
                    )
                    # adjacent 64-row rope matmuls occupy disjoint PE
                    # quadrants (rows 0-63 / 64-127) and overlap in the array
                    nc.tensor.matmul(
                        ps_a, t_kpe[0:64, ks], t_qpe[0:64, g, qs],
                        start=False, stop=True,
                    )
                    nc.tensor.matmul(
                        ps_b, t_kpe[64:128, ks], t_qpe[64:128, g, qs],
                        start=False, stop=True,
                    )
                    if diag:  # causal bias: -30000 * U_r over live columns
                        r = kb - 4 * qb
                        nc.tensor.matmul(
                            ps_a[:, : (r + 1) * 128],
                            t_negeye,
                            t_mask[:, r, : (r + 1) * 128],
                            start=False, stop=True,
                            skip_group_check=True,
                        )
                        nc.tensor.matmul(
                            ps_b[:, : (r + 1) * 128],
                            t_negeye,
                            t_mask[:, r, : (r + 1) * 128],
                            start=False, stop=True,
                            skip_group_check=True,
                        )
                    while pend_av:
                        emit_av(*pend_av.pop(0))
                    while pend_post and pend_post[0][1] <= idx:
                        emit_post(pend_post.pop(0)[0])
                    if pending_op:
                        pending_op.pop(0)()
                    ptp2 = ptp.tile([128, 2, 512], BF16, tag="pt", name="ptp2")
                    pt0 = ptp2[:, 0, :]
                    pt1 = ptp2[:, 1, :]
                    nc.scalar.activation(
                        ptp2[:, 0:2, :], ps_p[:, 0:2, :], AF.Exp, scale=SCALE
                    )
                    if kb == 0:
                        accs[h0] = smallp.tile(
                            [128, 512], F32R, tag="acc", name="acc"
                        )
                        accs[h1] = smallp.tile(
                            [128, 512], F32R, tag="acc", name="acc"
                        )
                        nc.vector.tensor_copy(accs[h0], pt0)
                        nc.vector.tensor_copy(accs[h1], pt1)
                    else:
                        nc.vector.tensor_tensor(accs[h0], accs[h0], pt0, op=OP.add)
                        nc.vector.tensor_tensor(accs[h1], accs[h1], pt1, op=OP.add)
                    pend_av.append((h0, kb, pt0))
                    pend_av.append((h1, kb, pt1))
                    if kb == nkb - 1:
                        pend_post.append((h0, idx + 1))
                        pend_post.append((h1, idx + 1))
                while pend_av:
                    emit_av(*pend_av.pop(0))
                while pend_post:
                    emit_post(pend_post.pop(0)[0])
                while pending_op:
                    pending_op.pop(0)()
                pending_op = [
                    make_chain(qb, mt, f) for mt in range(4) for f in range(4)
                ]
            while pending_op:
                pending_op.pop(0)()
    return nc


_NC_CACHE = {}


def build_mla(n=2048):
    if n not in _NC_CACHE:
        nc = bacc.Bacc(
            "TRN2",
            target_bir_lowering=False,
            debug=False,
            enable_asserts=False,
        )
        _emit(nc, n)
        nc.compile()
        _NC_CACHE[n] = nc
    return _NC_CACHE[n]


def make_host_inputs(x, Wqa, Wqb, Wkva, Wkvb, Wout, n):
    """Build the 8 per-core input maps (host-side sharding)."""
    # rope tables
    theta = BASE ** (-2.0 * np.arange(ROPE // 2, dtype=np.float32) / ROPE)
    pos = np.arange(n, dtype=np.float32)
    ang = pos[:, None] * theta[None, :]  # [n, 32]
    cos64 = np.repeat(np.cos(ang).T, 2, axis=0).astype(np.float32)  # [64, n]
    sin64 = np.repeat(np.sin(ang).T, 2, axis=0).astype(np.float32)
    cosd = np.tile(cos64, (2, 1))  # [128, n]
    sind = np.tile(sin64, (2, 1))

    kp = np.arange(128)[:, None, None]
    r = np.arange(4)[None, :, None]
    qf = np.arange(512)[None, None, :]
    # U_r: 1.0 where EXCLUDED (future) -> biased by -30000 before exp
    maskd = (qf < r * 128 + kp).astype(NPBF16)
    negeye = (MASK_BIAS * np.eye(128, dtype=np.float32)).astype(NPBF16)

    rot64 = np.zeros((64, 64), np.float32)
    for i in range(32):
        rot64[2 * i + 1, 2 * i] = -1.0
        rot64[2 * i, 2 * i + 1] = 1.0
    rotp = np.zeros((128, 128), np.float32)
    rotp[:64, :64] = rot64
    rotp[64:, 64:] = rot64
    rotk = np.hstack([rot64, rot64])
    eyek = np.hstack([np.eye(64, dtype=np.float32), np.eye(64, dtype=np.float32)])

    def prelay(w, kc):
        # [kc*128, m] -> [128, kc, m] partition-major, contiguous
        return np.ascontiguousarray(
            w.reshape(kc, 128, w.shape[1]).transpose(1, 0, 2)
        ).astype(NPBF16)

    def mmajor(w):
        # [128, KC, 4*128] -> [128, 4, KC, 128]
        return np.ascontiguousarray(
            w.reshape(128, KC, 4, 128).transpose(0, 2, 1, 3)
        )

    wkva_l = prelay(Wkva, KC)
    shared = {
        "wqa": mmajor(prelay(Wqa, KC).astype(np.float32)).astype(NPBF16),
        "wkva": mmajor(wkva_l[:, :, :512].astype(np.float32)).astype(NPBF16),
        "wkpe": np.ascontiguousarray(wkva_l[:, :, 512:576]),
        "cosd": cosd.astype(NPBF16),
        "sind": sind.astype(NPBF16),
        "maskd": maskd,
        "negeye": negeye,
        "onesd": np.ones((128, 1), np.float32),
        "rotp": rotp.astype(NPBF16),
        "rotk": rotk.astype(NPBF16),
        "eyek": eyek.astype(NPBF16),
    }
    Wqb_r = Wqb.reshape(512, HEADS, Q_HEAD)
    Wkvb_r = Wkvb.reshape(KV_RANK, HEADS, NOPE + VDIM)
    Wout_r = Wout.reshape(HEADS, VDIM, EMBED)

    in_maps = []
    TC = n // 512
    # x[be].T -> [128, TC, KC, 512]: f=(c,p), t=(tb,tt)
    xT = [
        np.ascontiguousarray(
            x[be].T.reshape(KC, 128, TC, 512).transpose(1, 2, 0, 3)
        ).astype(NPBF16)
        for be in range(x.shape[0])
    ]
    for c in range(8):
        be, hg = c // 4, c % 4
        hsel = slice(4 * hg, 4 * hg + NH)
        wqb = prelay(
            np.concatenate(
                [
                    Wqb_r[:, hsel, :NOPE].reshape(512, NH * NOPE),
                    Wqb_r[:, hsel, NOPE:].reshape(512, NH * ROPE),
                ],
                axis=1,
            ),
            RC,
        )
        in_maps.append(
            {
                **shared,
                "xT": xT[be],
                "wqb": wqb,
                "wk": prelay(Wkvb_r[:, hsel, :NOPE].reshape(512, NH * NOPE), RC),
                "wv": prelay(Wkvb_r[:, hsel, NOPE:].reshape(512, NH * VDIM), RC),
                "wout": prelay(Wout_r[hsel].reshape(NH * VDIM, EMBED), NH),
            }
        )
    return in_maps


def kernel(x, Wqa, Wqb, Wkva, Wkvb, Wout, _trace=False):
    x = np.asarray(x)
    b, n, _ = x.shape
    nc = build_mla(n)
    in_maps = make_host_inputs(
        np.asarray(x),
        np.asarray(Wqa),
        np.asarray(Wqb),
        np.asarray(Wkva),
        np.asarray(Wkvb),
        np.asarray(Wout),
        n,
    )
    res = bass_utils.run_bass_kernel_spmd(
        nc, in_maps, core_ids=list(range(8)), trace=_trace
    )
    out = np.zeros((b, n, EMBED), np.float32)
    for c in range(8):
        out[c // 4] += res.results[c]["out"].astype(np.float32)
    if _trace:
        kernel.last_results = res
    return out
